# revision 1
# baseline (speedup 1.0000x reference)
"""Multi-head causal self-attention on 8 Trainium2 NeuronCores.

Tensor-parallel over heads: core i owns heads (2i, 2i+1). bf16 matmul
operands throughout (fp32 PSUM accumulation); harness tolerance 2e-2.

Per core:
  phase 1: qT/kT/vT = (W_slice^T @ x^T) for its 2 heads; vT transposed
           on the PE into [token, d] tiles. Emission interleaves the
           second token-group's QKV with batch-0 attention to keep the
           PE dense (HAM stays un-throttled).
  phase 2: per (b, qj of 512 q, ki of 128 k), qj-outer: scoresT[k,q]
           for both heads as two row-tiled matmuls (head0 on PE rows
           0-63, head1 on rows 64-127, running concurrently); one Exp
           activation over the paired [128,1024] 2-bank PSUM tile (3D
           strided AP fuses the two heads on causal-diagonal blocks,
           which are narrowed to their valid columns + a [128,128]
           tril mask multiply). PV accumulates [V_h | ones*64]^T @
           attnT into po_h[128, 512]: rows 64:128 all hold the softmax
           denominator l, so normalization is copy(l)+recip+mul on DVE
           straight out of PSUM - no broadcast matmuls, inline per qj.
  phase 3: per b: AllToAll shards a_sb by 256-token chunks so core i
           ends up with [1024 features, 256 tokens] for tokens
           256i..256(i+1); local full W_proj^T @ A + bias -> outT.
           A2A(b0) flies under b1's attention; A2A(b1) under proj(b0).
           A tiny warmup AllToAll at kernel start absorbs the ~11us
           first-collective latency.
Host reassembles the 8 token chunks per batch.
"""

import numpy as np

B, T, C, H = 2, 2048, 1024, 16
D = C // H            # 64
NCORES = 8
HL = H // NCORES      # 2 heads per core
NT = B * T            # 4096
NQ = T // 512         # 4 q-blocks of 512 per b
NK = T // 128         # 16 k-chunks of 128 per b
TCH = T // NCORES     # 256-token chunk per core per b (A2A shard)
SCALE = float(D) ** -0.5

_cache = {}


def _build(mode: str):
    """mode: 'causal' | 'none' (all-ones mask)."""
    import concourse.mybir as mybir
    import concourse.tile as tile
    from concourse import bacc

    f32 = mybir.dt.float32
    f32r = mybir.dt.float32r
    mdt = mybir.dt.bfloat16

    nc = bacc.Bacc("TRN2", target_bir_lowering=False, debug=False,
                   num_devices=NCORES)
    xT = nc.dram_tensor("xT", [C, NT], mdt, kind="ExternalInput").ap()
    wqkv = nc.dram_tensor("wqkv", [C, 3 * HL * D], mdt,
                          kind="ExternalInput").ap()
    wp = nc.dram_tensor("wp", [C, C], mdt, kind="ExternalInput").ap()
    bias = nc.dram_tensor("bias", [128, NCORES], f32,
                          kind="ExternalInput").ap()
    cmask = nc.dram_tensor("cmask", [128, 384], mdt,
                           kind="ExternalInput").ap()
    onesv = nc.dram_tensor("onesv", [128, 64 * B * NK], mdt,
                           kind="ExternalInput").ap()
    outT = nc.dram_tensor("outT", [C, B * TCH], f32,
                          kind="ExternalOutput").ap()

    causal = mode == "causal"
    Exp = mybir.ActivationFunctionType.Exp

    with tile.TileContext(nc) as tc, \
         nc.allow_low_precision(reason="bf16 matmul path, tol 2e-2"):
        with tc.tile_pool(name="persist", bufs=1) as persist, \
             tc.tile_pool(name="dram", bufs=1, space="DRAM") as dram:
            q_sb = persist.tile([128, NT], mdt)
            k_sb = persist.tile([128, NT], mdt)
            # V^T tiles per head h: cols 128h:128h+64 = V_h d-columns,
            # cols 128h+64:128h+128 = ones (so PV output rows 64:128 all
            # hold the softmax denominator l, replicated for cheap DVE
            # normalization straight out of PSUM).
            vboth = persist.tile([128, 256, B * NK], mdt)
            cm_sb = persist.tile([128, 384], mdt)
            wqkv_sb = persist.tile([128, 8, 3 * HL * D], mdt)
            wp_sb = persist.tile([128, 8, C], mdt)
            bias_sb = persist.tile([128, NCORES], f32)
            a2a_in0 = dram.tile([NCORES * 128, TCH], mdt)
            a2a_in1 = dram.tile([NCORES * 128, TCH], mdt)
            a2a_out0 = dram.tile([NCORES * 128, TCH], mdt)
            a2a_out1 = dram.tile([NCORES * 128, TCH], mdt)
            a2a_ins = [a2a_in0, a2a_in1]
            a2a_outs = [a2a_out0, a2a_out1]
            warm_in = dram.tile([NCORES, 16], mdt)
            warm_out = dram.tile([NCORES, 16], mdt)

            nc.sync.dma_start(out=wqkv_sb[:],
                              in_=wqkv.rearrange("(a p) n -> p a n", p=128))
            nc.gpsimd.dma_start(out=cm_sb[:], in_=cmask[:])
            nc.gpsimd.dma_start(out=bias_sb[:], in_=bias[:])
            nc.gpsimd.dma_start(
                out=vboth[:, 64:128, :],
                in_=onesv.rearrange("p (c j) -> p c j", c=64))
            nc.gpsimd.dma_start(
                out=vboth[:, 192:256, :],
                in_=onesv.rearrange("p (c j) -> p c j", c=64))
            nc.gpsimd.dma_start(out=wp_sb[:],
                                in_=wp.rearrange("(a p) n -> p a n", p=128))
            ident = cm_sb[:, 256:384]

            # PSUM layout (8 banks):
            #   mm1 (2 banks): phase-1 qkv ps + v-transposes + norm rb +
            #                  proj pr, all via shared slot group
            #   sc  (4 banks): paired score tiles [128,1024]
            #   po  (2 banks): po_h0 / po_h1 accumulators
            with tc.tile_pool(name="mm1", bufs=2, space="PSUM") as mm1, \
                 tc.tile_pool(name="sc_psum", bufs=2, space="PSUM") as scp, \
                 tc.tile_pool(name="po_psum", bufs=1, space="PSUM") as pop, \
                 tc.tile_pool(name="xn_pool", bufs=2) as xp, \
                 tc.tile_pool(name="vtmp_pool", bufs=2) as vpool, \
                 tc.tile_pool(name="at_pool", bufs=6) as apool, \
                 tc.tile_pool(name="rb_pool", bufs=2) as rbp, \
                 tc.tile_pool(name="a_pool", bufs=2) as ap_pool, \
                 tc.tile_pool(name="agt_pool", bufs=2) as agp, \
                 tc.tile_pool(name="out_pool", bufs=3) as outp:

                def qkv_group(ng):
                    """QKV projection for token blocks ng*2048..(+2048)."""
                    xn = xp.tile([128, 8, 2048], mdt, tag="xn", name="xn")
                    for nl_ in range(4):
                        for kc in range(8):
                            nc.sync.dma_start(
                                out=xn[:, kc, nl_ * 512:(nl_ + 1) * 512],
                                in_=xT[kc * 128:(kc + 1) * 128,
                                       ng * 2048 + nl_ * 512:
                                       ng * 2048 + (nl_ + 1) * 512])
                    for nl in range(4):
                        n = ng * 4 + nl
                        tok = slice(n * 512, (n + 1) * 512)
                        for m in range(3):  # 0: q, 1: k, 2: v
                            ps = mm1.tile([128, 512], f32, tag="ps",
                                          name="ps")
                            for kc in range(8):
                                nc.tensor.matmul(
                                    ps[:],
                                    wqkv_sb[:, kc, m * 128:(m + 1) * 128],
                                    xn[:, kc, nl * 512:(nl + 1) * 512],
                                    start=(kc == 0), stop=(kc == 7))
                            if m == 0:
                                nc.vector.tensor_copy(q_sb[:, tok], ps[:])
                            elif m == 1:
                                nc.vector.tensor_copy(k_sb[:, tok], ps[:])
                            else:
                                vtmp = vpool.tile([128, 512], mdt,
                                                  tag="vtmp", name="vtmp")
                                nc.vector.tensor_copy(vtmp[:], ps[:])
                                bb = n // NQ
                                for s in range(4):
                                    j = bb * NK + (n % NQ) * 4 + s
                                    pt = mm1.tile([128, 128], mdt, tag="ps",
                                                  name="pt")
                                    nc.tensor.transpose(
                                        pt[:],
                                        vtmp[:, s * 128:(s + 1) * 128],
                                        ident)
                                    nc.vector.tensor_copy(
                                        vboth[:, 0:64, j], pt[:, 0:64])
                                    nc.vector.tensor_copy(
                                        vboth[:, 128:192, j],
                                        pt[:, 64:128])

                def attn_loops(b, a_sb, qjs):
                    """Score/exp/PV loops + normalization for batch b."""
                    for qj in qjs:
                        last_ki = 4 * qj + 3 if causal else NK - 1
                        po0 = pop.tile([128, 512], f32, tag="po0",
                                       name="po0")
                        po1 = pop.tile([128, 512], f32, tag="po1",
                                       name="po1")
                        pos = [po0, po1]
                        def emit_pv(ki, at, st):
                            vj = b * NK + ki
                            for h in range(2):
                                nc.tensor.matmul(
                                    pos[h][:, st:512],
                                    vboth[:, 128 * h:128 * h + 128, vj],
                                    at[:, 512 * h + st:512 * h + 512],
                                    start=(ki == 0), stop=(ki == last_ki))

                        # software-pipelined by one stage: scores(ki+1)
                        # sit ahead of PV(ki) in the PE FIFO so the PE
                        # never stalls on exp(ki)
                        pend = None
                        for ki in range(last_ki + 1):
                            diag = causal and ki >= 4 * qj
                            st = (ki - 4 * qj) * 128 if diag else 0
                            kc_ = slice(b * T + ki * 128,
                                        b * T + (ki + 1) * 128)
                            qc = slice(b * T + qj * 512 + st,
                                       b * T + (qj + 1) * 512)
                            sc = scp.tile([128, 1024], f32, tag="sc",
                                          name="sc")
                            nc.tensor.matmul(
                                sc[:, st:512], k_sb[0:64, kc_],
                                q_sb[0:64, qc], start=True, stop=True)
                            nc.tensor.matmul(
                                sc[:, 512 + st:1024], k_sb[64:128, kc_],
                                q_sb[64:128, qc], start=True, stop=True)
                            if pend is not None:
                                emit_pv(*pend)
                            at = apool.tile([128, 1024], mdt, tag="at",
                                            name="at")
                            if diag:
                                at3 = at[:].rearrange(
                                    "p (c t) -> p c t", c=2)[:, :, st:512]
                                sc3 = sc[:].rearrange(
                                    "p (c t) -> p c t", c=2)[:, :, st:512]
                                nc.scalar.activation(at3, sc3, Exp,
                                                     scale=SCALE)
                                atm = at[:].rearrange(
                                    "p (c t) -> p c t",
                                    c=2)[:, :, st:st + 128]
                                cm2 = cm_sb[:, 0:256].rearrange(
                                    "p (c t) -> p c t", c=2)
                                nc.vector.tensor_mul(atm, atm, cm2)
                            else:
                                nc.scalar.activation(at[:], sc[:], Exp,
                                                     scale=SCALE)
                            pend = (ki, at, st)
                        emit_pv(*pend)
                        # normalize: po rows 64:128 are l replicated
                        for h in range(2):
                            lrep = rbp.tile([64, 512], f32, tag="lrep",
                                            name="lrep")
                            nc.vector.tensor_copy(lrep[:],
                                                  pos[h][64:128, :])
                            rb = rbp.tile([64, 512], f32, tag="rb",
                                          name="rb")
                            nc.vector.reciprocal_approx_fast(rb[:],
                                                             lrep[:])
                            nc.vector.tensor_mul(
                                a_sb[64 * h:64 * h + 64,
                                     qj * 512:(qj + 1) * 512],
                                pos[h][0:64, :], rb[:])

                def stage_b(b, a_sb):
                    nc.sync.dma_start(
                        out=a2a_ins[b].rearrange("(c p) t -> p c t", p=128),
                        in_=a_sb[:].rearrange("p (c t) -> p c t",
                                              c=NCORES))

                def a2a_b(b):
                    nc.gpsimd.collective_compute(
                        "AllToAll", mybir.AluOpType.bypass,
                        replica_groups=[list(range(NCORES))],
                        ins=[a2a_ins[b].opt()], outs=[a2a_outs[b].opt()])

                def proj_b(b):
                    """Local projection for batch b's token chunk."""
                    agt = agp.tile([128, 8, TCH], mdt, tag="agt",
                                   name="agt")
                    nc.sync.dma_start(
                        out=agt[:],
                        in_=a2a_outs[b].rearrange("(c p) t -> p c t",
                                                  p=128))
                    for o in range(8):
                        pr = mm1.tile([128, TCH], f32, tag="ps", name="pr")
                        for kc in range(8):
                            nc.tensor.matmul(
                                pr[:], wp_sb[:, kc, o * 128:(o + 1) * 128],
                                agt[:, kc, :],
                                start=(kc == 0), stop=(kc == 7))
                        ot = outp.tile([128, TCH], f32, tag="ot", name="ot")
                        nc.vector.tensor_scalar_add(ot[:], pr[:],
                                                    bias_sb[:, o:o + 1])
                        nc.sync.dma_start(
                            out=outT[o * 128:(o + 1) * 128,
                                     b * TCH:(b + 1) * TCH],
                            in_=ot[:])

                # tiny warmup collective to absorb first-op CC latency
                nc.gpsimd.collective_compute(
                    "AllToAll", mybir.AluOpType.bypass,
                    replica_groups=[list(range(NCORES))],
                    ins=[warm_in.opt()], outs=[warm_out.opt()])
                a_sb0 = ap_pool.tile([128, T], mdt, tag="a_sb",
                                     name="a_sb0")
                a_sb1 = ap_pool.tile([128, T], mdt, tag="a_sb",
                                     name="a_sb1")
                qkv_group(0)
                attn_loops(0, a_sb0, [0, 1])
                qkv_group(1)
                attn_loops(0, a_sb0, [2, 3])
                stage_b(0, a_sb0)
                a2a_b(0)
                attn_loops(1, a_sb1, range(NQ))
                stage_b(1, a_sb1)
                a2a_b(1)
                # proj(b0) PE work hides A2A(b1) flight time
                proj_b(0)
                proj_b(1)

    nc.compile()
    return nc


def _get_program(mode: str):
    if mode not in _cache:
        _cache[mode] = _build(mode)
    return _cache[mode]


def kernel(**inputs):
    import ml_dtypes
    from concourse.bass_utils import run_bass_kernel_spmd

    bf16 = ml_dtypes.bfloat16

    x = np.asarray(inputs["x"], dtype=np.float32)
    mask = np.asarray(inputs["causal_mask"])
    Wqkv = np.asarray(inputs["W_qkv"], dtype=np.float32)
    Wp = np.asarray(inputs["W_proj"], dtype=np.float32)
    bp = np.asarray(inputs["b_proj"], dtype=np.float32)

    m2 = mask.reshape(T, T)
    if np.all(m2 != 0):
        mode = "none"
    else:
        tril = np.tril(np.ones((T, T), dtype=m2.dtype))
        if np.array_equal(m2, tril):
            mode = "causal"
        else:
            raise NotImplementedError("general mask not supported")

    nc = _get_program(mode)

    xT = np.ascontiguousarray(x.reshape(NT, C).T).astype(bf16)

    # [128,256]: triu mask (k<=q within a diagonal 128-block) | identity
    p = np.arange(128)[:, None]
    f = np.arange(128)[None, :]
    tri = (p <= f).astype(np.float32)
    cm = np.concatenate(
        [tri, tri, np.eye(128, dtype=np.float32)], axis=1).astype(bf16)

    Wq = Wqkv[:, 0 * C:1 * C]
    Wk = Wqkv[:, 1 * C:2 * C]
    Wv = Wqkv[:, 2 * C:3 * C]
    wp_bf = np.ascontiguousarray(Wp).astype(bf16)
    bias_h = np.ascontiguousarray(bp.reshape(NCORES, 128).T)

    in_maps = []
    for i in range(NCORES):
        hcols = slice(2 * i * D, (2 * i + 2) * D)  # this core's 2 heads
        wqkv_i = np.concatenate(
            [Wq[:, hcols], Wk[:, hcols], Wv[:, hcols]], axis=1)
        in_maps.append({
            "xT": xT,
            "wqkv": np.ascontiguousarray(wqkv_i).astype(bf16),
            "wp": wp_bf,
            "bias": bias_h,
            "cmask": cm,
            "onesv": np.ones((128, 64 * B * NK), dtype=bf16),
        })

    res = run_bass_kernel_spmd(nc, in_maps, list(range(NCORES)))

    out = np.empty((B, T, C), dtype=np.float32)
    for i in range(NCORES):
        oT = res.results[i]["outT"]  # [C, B*TCH] f32
        for b in range(B):
            out[b, i * TCH:(i + 1) * TCH, :] = \
                oT[:, b * TCH:(b + 1) * TCH].T
    return out



# revision 5
# speedup vs baseline: 1.0238x; 1.0238x over previous
"""Multi-head causal self-attention on 8 Trainium2 NeuronCores.

Tensor-parallel over heads: core i owns heads (2i, 2i+1). bf16 matmul
operands throughout (fp32 PSUM accumulation); harness tolerance 2e-2.

Per core (v2 — deep-pipelined emission):
  The whole kernel is one merged pipeline: attention for batch 0 starts
  as soon as the first 512-token QKV block exists, so the Exp activation
  engine (the attention-phase bottleneck at ~1.1us per ki step) ramps up
  ~15us earlier than a phase-split schedule, while the remaining QKV
  matmul chains fill the PE between score/PV work.

  qkv: per 512-token block, q/k/v = (W_slice^T @ x^T) for the core's 2
       heads; vT produced by DMA-XBAR transposes (off the PE).
  attn: per (b, qj of 512 q, ki of 128 k): scoresT[k,q] for both heads
       as two row-tiled K=64 matmuls running concurrently on PE rows
       0-63 / 64-127; causal diagonal blocks get an additive -1e5 mask
       via one extra [128,128] matmul per head accumulated into the
       score PSUM (no DVE in the exp->PV chain); one Exp over the
       paired [128,1024] PSUM (split into two 2D activations when the
       diagonal narrows — 3D strided APs run at half rate); PV
       accumulates [V_h | ones]^T @ attnT into po_h[128,512] whose rows
       64:128 hold the softmax denominator l; normalization is
       reciprocal straight out of PSUM + one multiply per head.
  a2a/proj: output resharded per (b, half-of-T) — 4 small AllToAlls
       pipelined under attention, each followed by a local
       W_proj^T @ A + bias on its [1024, 128]-token chunk, so the last
       collective only exposes ~one small chunk of latency at the tail.
       A tiny warmup AllToAll at kernel start absorbs the ~7-11us
       first-collective latency.
Host reassembles the 128-token chunks per (b, half).
"""

import numpy as np

B, T, C, H = 2, 2048, 1024, 16
D = C // H            # 64
NCORES = 8
HL = H // NCORES      # 2 heads per core
NT = B * T            # 4096
NQ = T // 512         # 4 q-blocks of 512 per b
NK = T // 128         # 16 k-chunks of 128 per b
SCALE = float(D) ** -0.5
MASKNEG = -1.0e5      # additive causal mask; exp(SCALE*(s+MASKNEG)) == 0

USE_DMA_TRANSPOSE = False
DIRECT_RECIP = False

_cache = {}


def _build(mode: str):
    """mode: 'causal' | 'none' (all-ones mask)."""
    import concourse.mybir as mybir
    import concourse.tile as tile
    from concourse import bacc

    f32 = mybir.dt.float32
    mdt = mybir.dt.bfloat16

    nc = bacc.Bacc("TRN2", target_bir_lowering=False, debug=False,
                   num_devices=NCORES)
    xT = nc.dram_tensor("xT", [C, NT], mdt, kind="ExternalInput").ap()
    # host-permuted: wqkv[p, kc*384 + m] = W_qkv_slice[kc*128 + p, m]
    wqkv = nc.dram_tensor("wqkv", [128, 8 * 3 * HL * D], mdt,
                          kind="ExternalInput").ap()
    # host-permuted: wp[p, kc*1024 + o] = W_proj[kc*128 + p, o]
    wp = nc.dram_tensor("wp", [128, 8 * C], mdt,
                        kind="ExternalInput").ap()
    bias = nc.dram_tensor("bias", [128, NCORES], f32,
                          kind="ExternalInput").ap()
    # cols 0:128 identity; cols 128:256 additive causal mask
    # (maskadd[r, c] = MASKNEG if r < c else 0)
    cmask = nc.dram_tensor("cmask", [128, 256], mdt,
                           kind="ExternalInput").ap()
    outT = nc.dram_tensor("outT", [C, B * 2 * 128], f32,
                          kind="ExternalOutput").ap()

    causal = mode == "causal"
    Exp = mybir.ActivationFunctionType.Exp

    with tile.TileContext(nc) as tc, \
         nc.allow_low_precision(reason="bf16 matmul path, tol 2e-2"):
        with tc.tile_pool(name="persist", bufs=1) as persist, \
             tc.tile_pool(name="dram", bufs=1, space="DRAM") as dram:
            q_sb = persist.tile([128, NT], mdt)
            k_sb = persist.tile([128, NT], mdt)
            # V^T tiles: vboth[p, h, j, 0:64] = V_h d-columns for k-chunk
            # j; vboth[p, h, j, 64:128] = ones (PV output rows 64:128 all
            # hold the softmax denominator l for cheap normalization).
            vboth = persist.tile([128, HL, B * NK, 128], mdt)
            cm_sb = persist.tile([128, 256], mdt)
            wqkv_sb = persist.tile([128, 8, 3 * HL * D], mdt)
            wp_sb = persist.tile([128, 8, C], mdt)
            bias_sb = persist.tile([128, NCORES], f32)
            a2a_ins = [dram.tile([NCORES * 128, 128], mdt,
                                 name=f"a2a_in{k}") for k in range(B * 2)]
            a2a_outs = [dram.tile([NCORES * 128, 128], mdt,
                                  name=f"a2a_out{k}") for k in range(B * 2)]
            warm_in = dram.tile([NCORES, 16], mdt)
            warm_out = dram.tile([NCORES, 16], mdt)

            nc.sync.dma_start(out=wqkv_sb[:],
                              in_=wqkv.rearrange("p (a n) -> p a n", a=8))
            nc.gpsimd.dma_start(out=cm_sb[:], in_=cmask[:])
            nc.gpsimd.dma_start(out=bias_sb[:], in_=bias[:])
            nc.gpsimd.dma_start(out=wp_sb[:],
                                in_=wp.rearrange("p (a n) -> p a n", a=8))
            for h in range(HL):
                nc.vector.memset(vboth[:, h, :, 64:128], 1.0)
            ident = cm_sb[:, 0:128]
            maskadd = cm_sb[:, 128:256]

            # PSUM layout (8 banks):
            #   mm1 (2 banks): qkv chains + proj accumulators
            #   sc  (4 banks): paired score tiles [128,1024] x2 in flight
            #   po  (2 banks): po_h0 / po_h1 accumulators
            with tc.tile_pool(name="mm1", bufs=2, space="PSUM") as mm1, \
                 tc.tile_pool(name="sc_psum", bufs=2, space="PSUM") as scp, \
                 tc.tile_pool(name="po_psum", bufs=1, space="PSUM") as pop, \
                 tc.tile_pool(name="xn_pool", bufs=2) as xp, \
                 tc.tile_pool(name="vtmp_pool", bufs=2) as vpool, \
                 tc.tile_pool(name="at_pool", bufs=6) as apool, \
                 tc.tile_pool(name="rb_pool", bufs=2) as rbp, \
                 tc.tile_pool(name="a_pool", bufs=2) as ap_pool, \
                 tc.tile_pool(name="agt_pool", bufs=2) as agp, \
                 tc.tile_pool(name="out_pool", bufs=3) as outp:

                xns = {}

                def xn_dmas(g, eng):
                    """Load x^T for token group g (2048 tokens)."""
                    xn = xp.tile([128, 8, 2048], mdt, tag="xn", name="xn")
                    xns[g] = xn
                    for nl_ in range(4):
                        for kc in range(8):
                            eng.dma_start(
                                out=xn[:, kc, nl_ * 512:(nl_ + 1) * 512],
                                in_=xT[kc * 128:(kc + 1) * 128,
                                       g * 2048 + nl_ * 512:
                                       g * 2048 + (nl_ + 1) * 512])

                def qkv_nl(g, nl):
                    """QKV projection for tokens g*2048+nl*512 .. +512."""
                    xn = xns[g]
                    n = g * 4 + nl
                    tok = slice(n * 512, (n + 1) * 512)
                    for m in range(3):  # 0: q, 1: k, 2: v
                        ps = mm1.tile([128, 512], f32, tag="ps", name="ps")
                        for kc in range(8):
                            nc.tensor.matmul(
                                ps[:],
                                wqkv_sb[:, kc, m * 128:(m + 1) * 128],
                                xn[:, kc, nl * 512:(nl + 1) * 512],
                                start=(kc == 0), stop=(kc == 7))
                        if m == 0:
                            nc.vector.tensor_copy(q_sb[:, tok], ps[:])
                        elif m == 1:
                            nc.vector.tensor_copy(k_sb[:, tok], ps[:])
                        else:
                            vtmp = vpool.tile([128, 512], mdt,
                                              tag="vtmp", name="vtmp")
                            nc.vector.tensor_copy(vtmp[:], ps[:])
                            bb = n // NQ
                            for s in range(4):
                                j = bb * NK + (n % NQ) * 4 + s
                                blk = vtmp[:, s * 128:(s + 1) * 128]
                                if USE_DMA_TRANSPOSE:
                                    nc.sync.dma_start_transpose(
                                        out=vboth[:, :, j, 0:64], in_=blk)
                                else:
                                    pt = mm1.tile([128, 128], mdt,
                                                  tag="ps", name="pt")
                                    nc.tensor.transpose(pt[:], blk, ident)
                                    nc.vector.tensor_copy(
                                        vboth[:, :, j, 0:64],
                                        pt[:].rearrange(
                                            "p (h d) -> p h d", h=2))

                def attn_qj(b, qj, a_sb):
                    """Score/exp/PV loop + normalization for (b, qj)."""
                    last_ki = 4 * qj + 3 if causal else NK - 1
                    po0 = pop.tile([128, 512], f32, tag="po0", name="po0")
                    po1 = pop.tile([128, 512], f32, tag="po1", name="po1")
                    pos = [po0, po1]

                    def emit_pv(ki, at, st):
                        vj = b * NK + ki
                        for h in range(2):
                            nc.tensor.matmul(
                                pos[h][:, st:512],
                                vboth[:, h, vj, :],
                                at[:, 512 * h + st:512 * h + 512],
                                start=(ki == 0), stop=(ki == last_ki))

                    # software-pipelined by one stage: scores(ki+1) sit
                    # ahead of PV(ki) in the PE FIFO so the PE never
                    # stalls on exp(ki)
                    pend = None
                    for ki in range(last_ki + 1):
                        diag = causal and ki >= 4 * qj
                        st = (ki - 4 * qj) * 128 if diag else 0
                        kc_ = slice(b * T + ki * 128,
                                    b * T + (ki + 1) * 128)
                        qc = slice(b * T + qj * 512 + st,
                                   b * T + (qj + 1) * 512)
                        sc = scp.tile([128, 1024], f32, tag="sc",
                                      name="sc")
                        nc.tensor.matmul(
                            sc[:, st:512], k_sb[0:64, kc_],
                            q_sb[0:64, qc], start=True, stop=not diag)
                        nc.tensor.matmul(
                            sc[:, 512 + st:1024], k_sb[64:128, kc_],
                            q_sb[64:128, qc], start=True, stop=not diag)
                        if diag:
                            # additive causal mask on the [128,128]
                            # diagonal block of each head
                            nc.tensor.matmul(
                                sc[:, st:st + 128], maskadd, ident,
                                start=False, stop=True)
                            nc.tensor.matmul(
                                sc[:, 512 + st:512 + st + 128], maskadd,
                                ident, start=False, stop=True)
                        if pend is not None:
                            emit_pv(*pend)
                        at = apool.tile([128, 1024], mdt, tag="at",
                                        name="at")
                        if st == 0:
                            nc.scalar.activation(at[:], sc[:], Exp,
                                                 scale=SCALE)
                        elif st <= 256:
                            # two contiguous 2D activations beat one
                            # half-rate 3D-strided one at these widths
                            for h in range(2):
                                nc.scalar.activation(
                                    at[:, 512 * h + st:512 * h + 512],
                                    sc[:, 512 * h + st:512 * h + 512],
                                    Exp, scale=SCALE)
                        else:
                            at3 = at[:].rearrange(
                                "p (c t) -> p c t", c=2)[:, :, st:512]
                            sc3 = sc[:].rearrange(
                                "p (c t) -> p c t", c=2)[:, :, st:512]
                            nc.scalar.activation(at3, sc3, Exp,
                                                 scale=SCALE)
                        pend = (ki, at, st)
                    emit_pv(*pend)
                    # normalize: po rows 64:128 are l replicated
                    for h in range(2):
                        rb = rbp.tile([64, 512], f32, tag="rb", name="rb")
                        if DIRECT_RECIP:
                            nc.vector.reciprocal_approx_fast(
                                rb[:], pos[h][64:128, :])
                        else:
                            lrep = rbp.tile([64, 512], f32, tag="lrep",
                                            name="lrep")
                            nc.vector.tensor_copy(lrep[:],
                                                  pos[h][64:128, :])
                            nc.vector.reciprocal_approx_fast(rb[:],
                                                             lrep[:])
                        nc.vector.tensor_mul(
                            a_sb[64 * h:64 * h + 64,
                                 qj * 512:(qj + 1) * 512],
                            pos[h][0:64, :], rb[:])

                def stage(b, hh, a_sb):
                    k4 = 2 * b + hh
                    nc.sync.dma_start(
                        out=a2a_ins[k4][:].rearrange("(c p) t -> p c t",
                                                     p=128),
                        in_=a_sb[:, hh * 1024:(hh + 1) * 1024].rearrange(
                            "p (c t) -> p c t", c=NCORES))

                def a2a(b, hh):
                    k4 = 2 * b + hh
                    nc.gpsimd.collective_compute(
                        "AllToAll", mybir.AluOpType.bypass,
                        replica_groups=[list(range(NCORES))],
                        ins=[a2a_ins[k4].opt()], outs=[a2a_outs[k4].opt()])

                agts = {}

                def agt_load(b, hh):
                    k4 = 2 * b + hh
                    agt = agp.tile([128, 8, 128], mdt, tag="agt",
                                   name="agt")
                    agts[k4] = agt
                    nc.sync.dma_start(
                        out=agt[:],
                        in_=a2a_outs[k4][:].rearrange("(c p) t -> p c t",
                                                      p=128))

                def proj(b, hh):
                    """Local projection for this core's 128-token chunk
                    of (b, half hh)."""
                    k4 = 2 * b + hh
                    agt = agts[k4]
                    for o in range(8):
                        pr = mm1.tile([128, 128], f32, tag="ps",
                                      name="pr")
                        for kc in range(8):
                            nc.tensor.matmul(
                                pr[:], wp_sb[:, kc, o * 128:(o + 1) * 128],
                                agt[:, kc, :],
                                start=(kc == 0), stop=(kc == 7))
                        ot = outp.tile([128, 128], f32, tag="ot",
                                       name="ot")
                        nc.vector.tensor_scalar_add(ot[:], pr[:],
                                                    bias_sb[:, o:o + 1])
                        nc.sync.dma_start(
                            out=outT[o * 128:(o + 1) * 128,
                                     k4 * 128:(k4 + 1) * 128],
                            in_=ot[:])

                # tiny warmup collective to absorb first-op CC latency
                nc.gpsimd.collective_compute(
                    "AllToAll", mybir.AluOpType.bypass,
                    replica_groups=[list(range(NCORES))],
                    ins=[warm_in.opt()], outs=[warm_out.opt()])
                a_sb0 = ap_pool.tile([128, T], mdt, tag="a_sb",
                                     name="a_sb0")
                a_sb1 = ap_pool.tile([128, T], mdt, tag="a_sb",
                                     name="a_sb1")
                xn_dmas(0, nc.sync)
                qkv_nl(0, 0)
                attn_qj(0, 0, a_sb0)
                qkv_nl(0, 1)
                attn_qj(0, 1, a_sb0)
                stage(0, 0, a_sb0)
                a2a(0, 0)
                qkv_nl(0, 2)
                xn_dmas(1, nc.scalar)
                attn_qj(0, 2, a_sb0)
                qkv_nl(0, 3)
                attn_qj(0, 3, a_sb0)
                stage(0, 1, a_sb0)
                a2a(0, 1)
                agt_load(0, 0)
                qkv_nl(1, 0)
                qkv_nl(1, 1)
                attn_qj(1, 0, a_sb1)
                proj(0, 0)
                qkv_nl(1, 2)
                attn_qj(1, 1, a_sb1)
                agt_load(0, 1)
                proj(0, 1)
                qkv_nl(1, 3)
                attn_qj(1, 2, a_sb1)
                stage(1, 0, a_sb1)
                a2a(1, 0)
                attn_qj(1, 3, a_sb1)
                stage(1, 1, a_sb1)
                a2a(1, 1)
                agt_load(1, 0)
                proj(1, 0)
                agt_load(1, 1)
                proj(1, 1)

    nc.compile()
    return nc


def _get_program(mode: str):
    if mode not in _cache:
        _cache[mode] = _build(mode)
    return _cache[mode]


def kernel(**inputs):
    import ml_dtypes
    from concourse.bass_utils import run_bass_kernel_spmd

    bf16 = ml_dtypes.bfloat16

    x = np.asarray(inputs["x"], dtype=np.float32)
    mask = np.asarray(inputs["causal_mask"])
    Wqkv = np.asarray(inputs["W_qkv"], dtype=np.float32)
    Wp = np.asarray(inputs["W_proj"], dtype=np.float32)
    bp = np.asarray(inputs["b_proj"], dtype=np.float32)

    m2 = mask.reshape(T, T)
    if np.all(m2 != 0):
        mode = "none"
    else:
        tril = np.tril(np.ones((T, T), dtype=m2.dtype))
        if np.array_equal(m2, tril):
            mode = "causal"
        else:
            raise NotImplementedError("general mask not supported")

    nc = _get_program(mode)

    xT = np.ascontiguousarray(x.reshape(NT, C).T).astype(bf16)

    # [128,256]: identity | additive causal mask (-1e5 above diagonal)
    p = np.arange(128)[:, None]
    f = np.arange(128)[None, :]
    madd = np.where(p < f, MASKNEG, 0.0).astype(np.float32)
    cm = np.concatenate(
        [np.eye(128, dtype=np.float32), madd], axis=1).astype(bf16)

    Wq = Wqkv[:, 0 * C:1 * C]
    Wk = Wqkv[:, 1 * C:2 * C]
    Wv = Wqkv[:, 2 * C:3 * C]
    # wp permuted so DMA lines are contiguous: [128, kc, o]
    wp_bf = np.ascontiguousarray(
        Wp.reshape(8, 128, C).transpose(1, 0, 2).reshape(128, 8 * C)
    ).astype(bf16)
    bias_h = np.ascontiguousarray(bp.reshape(NCORES, 128).T)

    in_maps = []
    for i in range(NCORES):
        hcols = slice(2 * i * D, (2 * i + 2) * D)  # this core's 2 heads
        wqkv_i = np.concatenate(
            [Wq[:, hcols], Wk[:, hcols], Wv[:, hcols]], axis=1)
        wqkv_p = np.ascontiguousarray(
            wqkv_i.reshape(8, 128, 3 * HL * D).transpose(1, 0, 2)
            .reshape(128, 8 * 3 * HL * D)).astype(bf16)
        in_maps.append({
            "xT": xT,
            "wqkv": wqkv_p,
            "wp": wp_bf,
            "bias": bias_h,
            "cmask": cm,
        })

    res = run_bass_kernel_spmd(nc, in_maps, list(range(NCORES)))

    out = np.empty((B, T, C), dtype=np.float32)
    for i in range(NCORES):
        oT = res.results[i]["outT"]  # [C, B*2*128] f32
        for b in range(B):
            for hh in range(2):
                k4 = 2 * b + hh
                out[b, hh * 1024 + i * 128:hh * 1024 + (i + 1) * 128, :] \
                    = oT[:, k4 * 128:(k4 + 1) * 128].T
    return out
